# revision 35
# baseline (speedup 1.0000x reference)
"""GCN (GCNConv + relu + dense + relu) on 8 NeuronCores — gpsimd ap_gather
edge engine.

Single SPMD launch. Nodes sharded by destination (12500/core, 4 streams of
3125 dests on 32-partition bands). Per core:

  table build:  g[128 bands*feats, node] = W_stack^T @ x_pre (x_pre is
                dinv-scaled, transposed, bf16, host-prepped), streamed in
                10 windows of 10000 nodes (ping-pong SBUF).
  edge gather:  one gpsimd.ap_gather per half-window: per (dest, stream,
                window) a fixed budget of uniform slot-3 gathers (regions
                R1 always / R2 if c>=4 / R3 if c>=7 / spill c>=10), pads
                point at a zero column.
  reduce:       DVE tensor_reduce [128, slots, 3] -> slot sums.
  cascade:      ap_gather R3->R2 positions, R2->canonical; DVE adds into
                the f32 accumulator [128, 3136] (col j = dest j of each
                band's 3125-dest range).
  spill:        per-window spill slot sums buffered; a few full-width
                ap_gather rounds at the end fold them in.
  epilogue:     acc*dinv_dst, +b_gcn, relu (Act), block-diag W_dense
                matmul (PE), +b_dense, relu, PE transpose, one DMA out.

Host does only layout/indexing prep (sharding, slot assignment, dinv
scaling/transpose of x) — all O(E) numpy; the model math runs on device.
"""

import sys

if "/opt/trn_rl_repo" not in sys.path:
    sys.path.insert(0, "/opt/trn_rl_repo")

import numpy as np
import ml_dtypes

import concourse.bacc as bacc
import concourse.mybir as mybir
from concourse import tile, library_config
from concourse.bass_utils import run_bass_kernel_spmd

# ------------------------------------------------------------- constants
N = 100000
E = 3200000
IN_DIM = 128
F = 32
NCORES = 8
NLOC = N // NCORES            # 12500
NSTR = 4                      # streams (32-partition bands)
DPS = NLOC // NSTR            # 3125 dests per stream
W = 10000                     # window (nodes)
NW = N // W                   # 10
SLOT = 3
R1 = 3136                     # canonical slots (>= DPS, %16-friendly)
R2 = 1328
R3 = 176
SP = 16
NSLOTS = R1 + R2 + R3 + SP    # 5008
NIDX = NSLOTS * SLOT          # 15024
H1S = 2336                    # half-split of NSLOTS; H1S % 32 == 0 so the
H2S = NSLOTS - H1S            # second idx slice stays 4B-aligned
Z_WIN = W                     # zero col in g window
Z_SS = NSLOTS                 # zero col in slot sums
Z_SP_BUF = NW * SP            # zero col in spill accumulator (480)
Z_CA = R3 + SP                # zero idx for cascade A in-window (368)
Z_CB = R2 + R3 + SP           # zero idx for cascade B in-window (1872)
OUTR = 3200                   # padded cols for output transpose (25*128)

BF16 = ml_dtypes.bfloat16


def _wrap(a):
    """[..., NSTR, n] -> [..., 128, n//16] int16: stream s duplicated onto
    groups 2s, 2s+1; idx j at partition 16g + j%16, free j//16."""
    n = a.shape[-1]
    assert n % 16 == 0
    lead = a.shape[:-2]
    b = a.reshape(lead + (NSTR, n // 16, 16))
    b = np.swapaxes(b, -1, -2)                      # [..., NSTR, 16, n//16]
    b = np.repeat(b, 2, axis=-3)                    # [..., 8, 16, n//16]
    return np.ascontiguousarray(
        b.reshape(lead + (128, n // 16)), dtype=np.int16
    )


def _wrap_subgathers(a, splits):
    """Wrap each sub-gather's idx range independently, concat along free."""
    outs = []
    off = 0
    for n in splits:
        outs.append(_wrap(a[..., off : off + n]))
        off += n
    assert off == a.shape[-1]
    return np.concatenate(outs, axis=-1)


# ------------------------------------------------------------- program


def build_nc(n_rounds, num_devices=NCORES, dumps=False):
    nc = bacc.Bacc(
        "TRN2", target_bir_lowering=False, debug=False, num_devices=num_devices
    )
    f32, i16, bf16 = mybir.dt.float32, mybir.dt.int16, mybir.dt.bfloat16
    if dumps:
        gwD = nc.dram_tensor("gwD", [128, W + 1], f32, kind="ExternalOutput")
        ssD = nc.dram_tensor("ssD", [NW, 128, NSLOTS + 1], f32, kind="ExternalOutput")
        spD = nc.dram_tensor("spD", [128, NW * SP + 1], f32, kind="ExternalOutput")
        accD = nc.dram_tensor("accD", [128, R1], f32, kind="ExternalOutput")

    xp_d = nc.dram_tensor("xp", [128, N], bf16, kind="ExternalInput")
    xo_d = nc.dram_tensor("xo", [128, NLOC], bf16, kind="ExternalInput")
    ws_d = nc.dram_tensor("ws", [128, 128], bf16, kind="ExternalInput")
    wd_d = nc.dram_tensor("wd", [128, 128], f32, kind="ExternalInput")
    bg_d = nc.dram_tensor("bg", [128, 1], f32, kind="ExternalInput")
    bd_d = nc.dram_tensor("bd", [128, 1], f32, kind="ExternalInput")
    eye_d = nc.dram_tensor("eye", [128, 128], f32, kind="ExternalInput")
    dv_d = nc.dram_tensor("dv", [128, R1], f32, kind="ExternalInput")
    im_d = nc.dram_tensor("im", [NW, 128, NIDX // 16], i16, kind="ExternalInput")
    ia_d = nc.dram_tensor("ia", [NW, 128, R2 // 16], i16, kind="ExternalInput")
    ib_d = nc.dram_tensor("ib", [NW, 128, R1 // 16], i16, kind="ExternalInput")
    isp_d = nc.dram_tensor(
        "isp", [max(n_rounds, 1), 128, R1 // 16], i16, kind="ExternalInput"
    )
    out_d = nc.dram_tensor("out", [OUTR * NSTR, F], f32, kind="ExternalOutput")

    with tile.TileContext(nc) as tc:
        with (
            tc.tile_pool(name="const", bufs=1) as cpool,
            tc.tile_pool(name="persist", bufs=1) as ppool,
            tc.tile_pool(name="psA", bufs=2, space="PSUM") as psA,
            tc.tile_pool(name="psB", bufs=2, space="PSUM") as psB,
        ):
            nc.gpsimd.load_library(library_config.ap_gather)

            ws_t = cpool.tile([128, 128], bf16)
            bg_t = cpool.tile([128, 1], f32)
            bd_t = cpool.tile([128, 1], f32)
            for t, d in [(ws_t, ws_d), (bg_t, bg_d), (bd_t, bd_d)]:
                nc.sync.dma_start(out=t[:], in_=d[:])

            ss_t = ppool.tile([128, NSLOTS + 1], f32)
            acc_t = ppool.tile([128, R1], f32)
            sp_t = ppool.tile([128, NW * SP + 1], f32)
            nc.vector.memset(ss_t[:, Z_SS : Z_SS + 1], 0.0)
            nc.vector.memset(acc_t[:], 0.0)
            nc.vector.memset(sp_t[:, Z_SP_BUF : Z_SP_BUF + 1], 0.0)

            with (
                tc.tile_pool(name="xw", bufs=4) as xpool,
                tc.tile_pool(name="gw", bufs=2) as gpool,
                tc.tile_pool(name="go", bufs=2) as opool,
                tc.tile_pool(name="casc", bufs=1) as capool,
                tc.tile_pool(name="idx", bufs=2) as ipool,
            ):
                # self-loop term first (overlaps window-0 build)
                for s in range(NSTR):
                    for off in range(0, DPS, 500):
                        n = min(500, DPS - off)
                        xoc_t = xpool.tile([128, 500], bf16, tag="x")
                        nc.sync.dma_start(
                            out=xoc_t[:, 0:n],
                            in_=xo_d[:, s * DPS + off : s * DPS + off + n],
                        )
                        op_t = psA.tile([128, 500], f32, tag="gp")
                        nc.tensor.matmul(
                            op_t[:, 0:n], ws_t[:], xoc_t[:, 0:n],
                            start=True, stop=True,
                        )
                        nc.vector.tensor_tensor(
                            out=acc_t[s * F : (s + 1) * F, off : off + n],
                            in0=acc_t[s * F : (s + 1) * F, off : off + n],
                            in1=op_t[s * F : (s + 1) * F, 0:n],
                            op=mybir.AluOpType.add,
                        )
                def build_win(w):
                    gw_t = gpool.tile([128, W + 1], f32, tag="gw", name="gw")
                    nc.vector.memset(gw_t[:, W : W + 1], 0.0)
                    for k in range(W // 500):
                        xc_t = xpool.tile([128, 500], bf16, tag="x")
                        nc.sync.dma_start(
                            out=xc_t[:],
                            in_=xp_d[:, w * W + k * 500 : w * W + (k + 1) * 500],
                        )
                        gp_t = psA.tile([128, 500], f32, tag="gp")
                        nc.tensor.matmul(
                            gp_t[:], ws_t[:], xc_t[:], start=True, stop=True
                        )
                        nc.scalar.activation(
                            gw_t[:, k * 500 : (k + 1) * 500],
                            gp_t[:],
                            mybir.ActivationFunctionType.Copy,
                        )
                    im_t = ipool.tile([128, NIDX // 16], i16, tag="im")
                    ia_t = ipool.tile([128, R2 // 16], i16, tag="ia")
                    ib_t = ipool.tile([128, R1 // 16], i16, tag="ib")
                    nc.sync.dma_start(out=im_t[:], in_=im_d[w])
                    nc.sync.dma_start(out=ia_t[:], in_=ia_d[w])
                    nc.sync.dma_start(out=ib_t[:], in_=ib_d[w])
                    return gw_t, im_t, ia_t, ib_t

                def gather_half(gw_t, im_t, s0, ns):
                    go_t = opool.tile(
                        [128, max(H1S, H2S) * SLOT], f32, tag="go"
                    )
                    nidx = ns * SLOT
                    nc.gpsimd.ap_gather(
                        go_t[:, 0:nidx].unsqueeze(2),
                        gw_t[:].unsqueeze(2),
                        im_t[:, s0 * SLOT // 16 : (s0 + ns) * SLOT // 16],
                        channels=128,
                        num_elems=W + 1,
                        d=1,
                        num_idxs=nidx,
                    )
                    return go_t

                def slot_adds(go_t, s0, ns):
                    v = go_t[:, 0 : ns * SLOT].rearrange(
                        "p (n s) -> p n s", n=ns
                    )
                    nc.vector.tensor_tensor(
                        out=ss_t[:, s0 : s0 + ns].unsqueeze(2),
                        in0=v[:, :, 0:1],
                        in1=v[:, :, 1:2],
                        op=mybir.AluOpType.add,
                    )
                    nc.vector.tensor_tensor(
                        out=ss_t[:, s0 : s0 + ns].unsqueeze(2),
                        in0=ss_t[:, s0 : s0 + ns].unsqueeze(2),
                        in1=v[:, :, 2:3],
                        op=mybir.AluOpType.add,
                    )

                def cascades(w, ia_t, ib_t):
                    ca_t = capool.tile([128, R2], f32, tag="ca")
                    nc.gpsimd.ap_gather(
                        ca_t[:].unsqueeze(2),
                        ss_t[:, R1 + R2 : NSLOTS + 1].unsqueeze(2),
                        ia_t[:],
                        channels=128,
                        num_elems=R3 + SP + 1,
                        d=1,
                        num_idxs=R2,
                    )
                    nc.vector.tensor_tensor(
                        out=ss_t[:, R1 : R1 + R2],
                        in0=ss_t[:, R1 : R1 + R2],
                        in1=ca_t[:],
                        op=mybir.AluOpType.add,
                    )
                    cb_t = capool.tile([128, R1], f32, tag="cb")
                    nc.gpsimd.ap_gather(
                        cb_t[:].unsqueeze(2),
                        ss_t[:, R1 : NSLOTS + 1].unsqueeze(2),
                        ib_t[:],
                        channels=128,
                        num_elems=R2 + R3 + SP + 1,
                        d=1,
                        num_idxs=R1,
                    )
                    nc.vector.tensor_tensor(
                        out=acc_t[:], in0=acc_t[:], in1=ss_t[:, 0:R1],
                        op=mybir.AluOpType.add,
                    )
                    nc.vector.tensor_tensor(
                        out=acc_t[:], in0=acc_t[:], in1=cb_t[:],
                        op=mybir.AluOpType.add,
                    )

                # software pipeline: g1 of window w+1 is issued on Pool
                # before window w's cascades (its inputs are ready), but its
                # DVE slot-adds run after window w's acc reads (DVE is
                # in-order, so ss is not clobbered early).
                cur = build_win(0)
                go1 = gather_half(cur[0], cur[1], 0, H1S)
                slot_adds(go1, 0, H1S)
                for w in range(NW):
                    gw_t, im_t, ia_t, ib_t = cur
                    go2 = gather_half(gw_t, im_t, H1S, H2S)
                    slot_adds(go2, H1S, H2S)
                    nc.vector.tensor_copy(
                        out=sp_t[:, w * SP : (w + 1) * SP],
                        in_=ss_t[:, R1 + R2 + R3 : NSLOTS],
                    )
                    nxt_go1 = None
                    if w + 1 < NW:
                        cur = build_win(w + 1)
                        nxt_go1 = gather_half(cur[0], cur[1], 0, H1S)
                    cascades(w, ia_t, ib_t)
                    if nxt_go1 is not None:
                        slot_adds(nxt_go1, 0, H1S)

                # ---- spill rounds
                for r in range(n_rounds):
                    is_t = ipool.tile([128, R1 // 16], i16, tag="isp")
                    nc.sync.dma_start(out=is_t[:], in_=isp_d[r])
                    sg_t = capool.tile([128, R1], f32, tag="cb")
                    nc.gpsimd.ap_gather(
                        sg_t[:].unsqueeze(2),
                        sp_t[:].unsqueeze(2),
                        is_t[:],
                        channels=128,
                        num_elems=NW * SP + 1,
                        d=1,
                        num_idxs=R1,
                    )
                    nc.vector.tensor_tensor(
                        out=acc_t[:], in0=acc_t[:], in1=sg_t[:],
                        op=mybir.AluOpType.add,
                    )

            if dumps:
                nc.sync.dma_start(out=spD[:], in_=sp_t[:])
                nc.sync.dma_start(out=accD[:], in_=acc_t[:])
            # ------------------------------------------------ epilogue
            with tc.tile_pool(name="epi", bufs=1) as epool:
                wd_t = epool.tile([128, 128], f32)
                eye_t = epool.tile([128, 128], f32)
                dv_t = epool.tile([128, R1], f32)
                nc.sync.dma_start(out=wd_t[:], in_=wd_d[:])
                nc.sync.dma_start(out=eye_t[:], in_=eye_d[:])
                nc.sync.dma_start(out=dv_t[:], in_=dv_d[:])
                nc.vector.tensor_tensor(
                    out=acc_t[:], in0=acc_t[:], in1=dv_t[:],
                    op=mybir.AluOpType.mult,
                )
                h1_t = epool.tile([128, R1], f32)
                nc.scalar.activation(
                    h1_t[:], acc_t[:],
                    mybir.ActivationFunctionType.Relu, bias=bg_t[:],
                )
                h2_t = epool.tile([128, OUTR], f32)
                nc.vector.memset(h2_t[:, R1:OUTR], 0.0)
                for k in range(R1 // 448):  # 3136 = 7*448
                    hp_t = psA.tile([128, 448], f32, tag="hp")
                    nc.tensor.matmul(
                        hp_t[:], wd_t[:],
                        h1_t[:, k * 448 : (k + 1) * 448],
                        start=True, stop=True,
                    )
                    nc.scalar.activation(
                        h2_t[:, k * 448 : (k + 1) * 448], hp_t[:],
                        mybir.ActivationFunctionType.Relu, bias=bd_t[:],
                    )
                ob_t = epool.tile([128, OUTR], f32)
                for t in range(OUTR // 128):
                    tp_t = psB.tile([128, 128], f32, tag="tp")
                    nc.tensor.transpose(
                        tp_t[:], h2_t[:, t * 128 : (t + 1) * 128], eye_t[:]
                    )
                    nc.scalar.activation(
                        ob_t[:, t * 128 : (t + 1) * 128], tp_t[:],
                        mybir.ActivationFunctionType.Copy,
                    )
                # out rows: [t, p, s, f] = ob[p, t*128 + s*32 + f]
                nc.sync.dma_start(
                    out=out_d.ap().rearrange(
                        "(t p s) f -> p t s f", p=128, s=NSTR
                    ),
                    in_=ob_t[:].rearrange("p (t s f) -> p t s f", s=NSTR, f=F),
                )
    nc.compile()
    return nc


# ------------------------------------------------------------- host prep


def host_prep(x, edge_index, W_gcn, b_gcn, W_dense, b_dense):
    x = np.ascontiguousarray(x, dtype=np.float32)
    W_gcn = np.ascontiguousarray(W_gcn, dtype=np.float32)
    W_dense = np.ascontiguousarray(W_dense, dtype=np.float32)
    b_gcn = np.ascontiguousarray(b_gcn, dtype=np.float32)
    b_dense = np.ascontiguousarray(b_dense, dtype=np.float32)
    row = np.asarray(edge_index[0], dtype=np.int64)
    col = np.asarray(edge_index[1], dtype=np.int64)

    deg = np.bincount(col, minlength=N).astype(np.float64) + 1.0
    dinv = (1.0 / np.sqrt(deg)).astype(np.float32)

    # self-loops are NOT routed through the gather: their contribution
    # (table[v] itself) is recomputed in the epilogue from xo = own x slice.

    # shared tensors
    xp = np.ascontiguousarray((x * dinv[:, None]).T.astype(BF16))  # [128, N]
    ws = np.zeros((128, 128), dtype=np.float32)
    for s in range(NSTR):
        ws[:, s * F : (s + 1) * F] = W_gcn
    ws = ws.astype(BF16)
    wd = np.zeros((128, 128), dtype=np.float32)
    for s in range(NSTR):
        wd[s * F : (s + 1) * F, s * F : (s + 1) * F] = W_dense
    bg = np.tile(b_gcn.reshape(F), NSTR).reshape(128, 1).astype(np.float32)
    bd = np.tile(b_dense.reshape(F), NSTR).reshape(128, 1).astype(np.float32)
    eye = np.eye(128, dtype=np.float32)

    # per-edge decomposition
    core = col // NLOC
    rem = col % NLOC
    stream = rem // DPS
    j = rem % DPS
    win = row // W
    soff = (row % W).astype(np.int64)

    # group edges by (core, stream, win, j)
    key = ((core * NSTR + stream) * NW + win) * DPS + j
    order = np.argsort(key, kind="stable")
    ks = key[order]
    so = soff[order]
    uq, starts, cnts = np.unique(ks, return_index=True, return_counts=True)
    nu = len(uq)
    ne = len(ks)
    rank = np.arange(ne) - np.repeat(starts, cnts)
    level = rank // SLOT
    pos = rank % SLOT

    grp = uq // DPS                  # (core, stream, win) id per unique key
    ju = uq % DPS
    assert cnts.max() <= 9 + SP * SLOT, cnts.max()

    def region_rank(mask):
        """rank of masked uq entries within their grp (ascending j)."""
        g = grp[mask]
        nm = int(mask.sum())
        if nm == 0:
            return np.zeros(0, dtype=np.int64)
        _, gstart, gcnt = np.unique(g, return_index=True, return_counts=True)
        return np.arange(nm) - np.repeat(gstart, gcnt)

    m2 = cnts >= SLOT + 1
    m3 = cnts >= 2 * SLOT + 1
    msp = cnts >= 3 * SLOT + 1
    r2rank = np.full(nu, -1, dtype=np.int64)
    r3rank = np.full(nu, -1, dtype=np.int64)
    sprank = np.full(nu, -1, dtype=np.int64)
    r2rank[m2] = region_rank(m2)
    r3rank[m3] = region_rank(m3)
    # spill slots: per uq, ceil((cnt-9)/3) slots; base = cumsum within grp
    spslots = np.where(msp, -(-(cnts - 3 * SLOT) // SLOT), 0)
    spbase = np.full(nu, -1, dtype=np.int64)
    if msp.any():
        g = grp[msp]
        v = spslots[msp]
        _, gstart, gcnt = np.unique(g, return_index=True, return_counts=True)
        cs = np.cumsum(v) - v
        base_per_grp = cs[gstart]
        spbase[msp] = cs - np.repeat(base_per_grp, gcnt)
    # capacity asserts (per grp)
    ngrp = NCORES * NSTR * NW
    r2cnt = np.bincount(grp[m2], minlength=ngrp)
    r3cnt = np.bincount(grp[m3], minlength=ngrp)
    spcnt = np.bincount(
        grp[msp], weights=spslots[msp].astype(np.float64), minlength=ngrp
    ).astype(np.int64)
    assert r2cnt.max() <= R2, r2cnt.max()
    assert r3cnt.max() <= R3, r3cnt.max()
    assert spcnt.max() <= SP, spcnt.max()

    # per-edge slot id within the NSLOTS space
    uq_of_edge = np.repeat(np.arange(nu), cnts)
    slot = np.empty(ne, dtype=np.int64)
    l0 = level == 0
    l1 = level == 1
    l2 = level == 2
    l3 = level >= 3
    slot[l0] = ju[uq_of_edge[l0]]
    slot[l1] = R1 + r2rank[uq_of_edge[l1]]
    slot[l2] = R1 + R2 + r3rank[uq_of_edge[l2]]
    slot[l3] = (
        R1 + R2 + R3 + spbase[uq_of_edge[l3]] + (level[l3] - 3)
    )
    idxpos = slot * SLOT + pos

    # main gather idx array
    cu = uq // (NW * DPS * NSTR)
    wu = (uq // DPS) % NW
    su = (uq // (NW * DPS)) % NSTR
    cu_e = np.repeat(cu, cnts)
    wu_e = np.repeat(wu, cnts)
    su_e = np.repeat(su, cnts)
    im = np.full((NCORES, NW, NSTR, NIDX), Z_WIN, dtype=np.int16)
    im[cu_e, wu_e, su_e, idxpos] = so.astype(np.int16)

    # cascade A: for each R2 slot -> its dest's R3 slot (or zero)
    ia = np.full((NCORES, NW, NSTR, R2), Z_CA, dtype=np.int16)
    ia[cu[m2], wu[m2], su[m2], r2rank[m2]] = np.where(
        m3[m2], r3rank[m2], Z_CA
    ).astype(np.int16)
    # cascade B: canonical j -> R2 slot (or zero)
    ib = np.full((NCORES, NW, NSTR, R1), Z_CB, dtype=np.int16)
    ib[cu[m2], wu[m2], su[m2], ju[m2]] = r2rank[m2].astype(np.int16)

    # spill final rounds: per (core, stream, j) pooled spill slots
    isp_list = []
    if msp.any():
        su_s = su[msp]
        cu_s = cu[msp]
        wu_s = wu[msp]
        ju_s = ju[msp]
        nsl = spslots[msp]
        sb = spbase[msp]
        # expand each uq's spill slots
        reps = np.repeat(np.arange(len(nsl)), nsl)
        k = np.arange(len(reps)) - np.repeat(
            np.cumsum(nsl) - nsl, nsl
        )
        gpos = wu_s[reps] * SP + sb[reps] + k      # pos in spill buffer
        ckey = (cu_s[reps] * NSTR + su_s[reps]) * DPS + ju_s[reps]
        o2 = np.argsort(ckey, kind="stable")
        ck = ckey[o2]
        gp = gpos[o2]
        _, cst, ccn = np.unique(ck, return_index=True, return_counts=True)
        rr = np.arange(len(ck)) - np.repeat(cst, ccn)
        n_rounds = int(ccn.max())
        isp = np.full((NCORES, n_rounds, NSTR, R1), Z_SP_BUF, dtype=np.int16)
        isp[ck // (NSTR * DPS), rr, (ck // DPS) % NSTR, ck % DPS] = gp.astype(
            np.int16
        )
    else:
        n_rounds = 0
        isp = np.full((NCORES, 1, NSTR, R1), Z_SP_BUF, dtype=np.int16)

    # wrap idx arrays
    im_w = np.stack(
        [
            _wrap_subgathers(im[c], [H1S * SLOT, H2S * SLOT])
            for c in range(NCORES)
        ]
    )
    ia_w = np.stack([_wrap(ia[c]) for c in range(NCORES)])
    ib_w = np.stack([_wrap(ib[c]) for c in range(NCORES)])
    isp_w = np.stack([_wrap(isp[c]) for c in range(NCORES)])

    # dinv per dest in canonical layout
    dv = np.zeros((NCORES, 128, R1), dtype=np.float32)
    for c in range(NCORES):
        for s in range(NSTR):
            seg = dinv[c * NLOC + s * DPS : c * NLOC + (s + 1) * DPS]
            dv[c, s * F : (s + 1) * F, 0:DPS] = seg[None, :]

    in_maps = []
    for c in range(NCORES):
        in_maps.append(
            {
                "xp": xp,
                "xo": np.ascontiguousarray(xp[:, c * NLOC : (c + 1) * NLOC]),
                "ws": ws,
                "wd": wd,
                "bg": bg,
                "bd": bd,
                "eye": eye,
                "dv": dv[c],
                "im": im_w[c],
                "ia": ia_w[c],
                "ib": ib_w[c],
                "isp": isp_w[c],
            }
        )
    return in_maps, n_rounds


def assemble_out(results):
    """results[c]["out"]: [OUTR*NSTR, F] rows = [t, p, s, f] ->
    dest j = t*128+p of stream s."""
    outs = []
    for c in range(NCORES):
        o = results[c]["out"].reshape(OUTR // 128, 128, NSTR, F)
        o = o.transpose(2, 0, 1, 3).reshape(NSTR * OUTR, F)
        outs.append(
            np.concatenate(
                [o[s * OUTR : s * OUTR + DPS] for s in range(NSTR)]
            )
        )
    return np.concatenate(outs, axis=0)


_NC_CACHE = {}


def _get_nc(n_rounds, num_devices=NCORES):
    key = (n_rounds, num_devices)
    if key not in _NC_CACHE:
        _NC_CACHE[key] = build_nc(n_rounds, num_devices)
    return _NC_CACHE[key]


def kernel(x, edge_index, W_gcn, b_gcn, W_dense, b_dense):
    in_maps, n_rounds = host_prep(
        x, edge_index, W_gcn, b_gcn, W_dense, b_dense
    )
    nc = _get_nc(n_rounds)
    core_ids = list(range(NCORES))
    res = run_bass_kernel_spmd(nc, in_maps, core_ids).results
    return assemble_out(res)


# revision 36
# speedup vs baseline: 1.0002x; 1.0002x over previous
"""GCN (GCNConv + relu + dense + relu) on 8 NeuronCores — gpsimd ap_gather
edge engine.

Single SPMD launch. Nodes sharded by destination (12500/core, 4 streams of
3125 dests on 32-partition bands). Per core:

  table build:  g[128 bands*feats, node] = W_stack^T @ x_pre (x_pre is
                dinv-scaled, transposed, bf16, host-prepped), streamed in
                10 windows of 10000 nodes (ping-pong SBUF).
  edge gather:  one gpsimd.ap_gather per half-window: per (dest, stream,
                window) a fixed budget of uniform slot-3 gathers (regions
                R1 always / R2 if c>=4 / R3 if c>=7 / spill c>=10), pads
                point at a zero column.
  reduce:       DVE tensor_reduce [128, slots, 3] -> slot sums.
  cascade:      ap_gather R3->R2 positions, R2->canonical; DVE adds into
                the f32 accumulator [128, 3136] (col j = dest j of each
                band's 3125-dest range).
  spill:        per-window spill slot sums buffered; a few full-width
                ap_gather rounds at the end fold them in.
  epilogue:     acc*dinv_dst, +b_gcn, relu (Act), block-diag W_dense
                matmul (PE), +b_dense, relu, PE transpose, one DMA out.

Host does only layout/indexing prep (sharding, slot assignment, dinv
scaling/transpose of x) — all O(E) numpy; the model math runs on device.
"""

import sys

if "/opt/trn_rl_repo" not in sys.path:
    sys.path.insert(0, "/opt/trn_rl_repo")

import numpy as np
import ml_dtypes

import concourse.bacc as bacc
import concourse.mybir as mybir
from concourse import tile, library_config
from concourse.bass_utils import run_bass_kernel_spmd

# ------------------------------------------------------------- constants
N = 100000
E = 3200000
IN_DIM = 128
F = 32
NCORES = 8
NLOC = N // NCORES            # 12500
NSTR = 4                      # streams (32-partition bands)
DPS = NLOC // NSTR            # 3125 dests per stream
W = 10000                     # window (nodes)
NW = N // W                   # 10
SLOT = 3
R1 = 3136                     # canonical slots (>= DPS, %16-friendly)
R2 = 1328
R3 = 176
SP = 16
NSLOTS = R1 + R2 + R3 + SP    # 5008
NIDX = NSLOTS * SLOT          # 15024
H1S = 2336                    # half-split of NSLOTS; H1S % 32 == 0 so the
H2S = NSLOTS - H1S            # second idx slice stays 4B-aligned
Z_WIN = W                     # zero col in g window
Z_SS = NSLOTS                 # zero col in slot sums
Z_SP_BUF = NW * SP            # zero col in spill accumulator (480)
Z_CA = R3 + SP                # zero idx for cascade A in-window (368)
Z_CB = R2 + R3 + SP           # zero idx for cascade B in-window (1872)
OUTR = 3200                   # padded cols for output transpose (25*128)

BF16 = ml_dtypes.bfloat16


def _wrap(a):
    """[..., NSTR, n] -> [..., 128, n//16] int16: stream s duplicated onto
    groups 2s, 2s+1; idx j at partition 16g + j%16, free j//16."""
    n = a.shape[-1]
    assert n % 16 == 0
    lead = a.shape[:-2]
    b = a.reshape(lead + (NSTR, n // 16, 16))
    b = np.swapaxes(b, -1, -2)                      # [..., NSTR, 16, n//16]
    b = np.repeat(b, 2, axis=-3)                    # [..., 8, 16, n//16]
    return np.ascontiguousarray(
        b.reshape(lead + (128, n // 16)), dtype=np.int16
    )


def _wrap_subgathers(a, splits):
    """Wrap each sub-gather's idx range independently, concat along free."""
    outs = []
    off = 0
    for n in splits:
        outs.append(_wrap(a[..., off : off + n]))
        off += n
    assert off == a.shape[-1]
    return np.concatenate(outs, axis=-1)


# ------------------------------------------------------------- program


def build_nc(n_rounds, num_devices=NCORES, dumps=False):
    nc = bacc.Bacc(
        "TRN2", target_bir_lowering=False, debug=False, num_devices=num_devices
    )
    f32, i16, bf16 = mybir.dt.float32, mybir.dt.int16, mybir.dt.bfloat16
    if dumps:
        gwD = nc.dram_tensor("gwD", [128, W + 1], f32, kind="ExternalOutput")
        ssD = nc.dram_tensor("ssD", [NW, 128, NSLOTS + 1], f32, kind="ExternalOutput")
        spD = nc.dram_tensor("spD", [128, NW * SP + 1], f32, kind="ExternalOutput")
        accD = nc.dram_tensor("accD", [128, R1], f32, kind="ExternalOutput")

    xp_d = nc.dram_tensor("xp", [128, N], bf16, kind="ExternalInput")
    xo_d = nc.dram_tensor("xo", [128, NLOC], bf16, kind="ExternalInput")
    ws_d = nc.dram_tensor("ws", [128, 128], bf16, kind="ExternalInput")
    wd_d = nc.dram_tensor("wd", [128, 128], f32, kind="ExternalInput")
    bg_d = nc.dram_tensor("bg", [128, 1], f32, kind="ExternalInput")
    bd_d = nc.dram_tensor("bd", [128, 1], f32, kind="ExternalInput")
    eye_d = nc.dram_tensor("eye", [128, 128], f32, kind="ExternalInput")
    dv_d = nc.dram_tensor("dv", [128, R1], f32, kind="ExternalInput")
    im_d = nc.dram_tensor("im", [NW, 128, NIDX // 16], i16, kind="ExternalInput")
    ia_d = nc.dram_tensor("ia", [NW, 128, R2 // 16], i16, kind="ExternalInput")
    ib_d = nc.dram_tensor("ib", [NW, 128, R1 // 16], i16, kind="ExternalInput")
    isp_d = nc.dram_tensor(
        "isp", [max(n_rounds, 1), 128, R1 // 16], i16, kind="ExternalInput"
    )
    out_d = nc.dram_tensor("out", [OUTR * NSTR, F], f32, kind="ExternalOutput")

    with tile.TileContext(nc) as tc:
        with (
            tc.tile_pool(name="const", bufs=1) as cpool,
            tc.tile_pool(name="persist", bufs=1) as ppool,
            tc.tile_pool(name="psA", bufs=2, space="PSUM") as psA,
            tc.tile_pool(name="psB", bufs=2, space="PSUM") as psB,
        ):
            nc.gpsimd.load_library(library_config.ap_gather)

            ws_t = cpool.tile([128, 128], bf16)
            bg_t = cpool.tile([128, 1], f32)
            bd_t = cpool.tile([128, 1], f32)
            for t, d in [(ws_t, ws_d), (bg_t, bg_d), (bd_t, bd_d)]:
                nc.sync.dma_start(out=t[:], in_=d[:])

            ss_t = ppool.tile([128, NSLOTS + 1], f32)
            acc_t = ppool.tile([128, R1], f32)
            sp_t = ppool.tile([128, NW * SP + 1], f32)
            nc.vector.memset(ss_t[:, Z_SS : Z_SS + 1], 0.0)
            nc.vector.memset(acc_t[:], 0.0)
            nc.vector.memset(sp_t[:, Z_SP_BUF : Z_SP_BUF + 1], 0.0)

            with (
                tc.tile_pool(name="xw", bufs=4) as xpool,
                tc.tile_pool(name="gw", bufs=2) as gpool,
                tc.tile_pool(name="go", bufs=2) as opool,
                tc.tile_pool(name="casc", bufs=1) as capool,
                tc.tile_pool(name="idx", bufs=2) as ipool,
            ):
                # self-loop term first (overlaps window-0 build)
                for s in range(NSTR):
                    for off in range(0, DPS, 500):
                        n = min(500, DPS - off)
                        xoc_t = xpool.tile([128, 500], bf16, tag="x")
                        nc.sync.dma_start(
                            out=xoc_t[:, 0:n],
                            in_=xo_d[:, s * DPS + off : s * DPS + off + n],
                        )
                        op_t = psA.tile([128, 500], f32, tag="gp")
                        nc.tensor.matmul(
                            op_t[:, 0:n], ws_t[:], xoc_t[:, 0:n],
                            start=True, stop=True,
                        )
                        nc.vector.tensor_tensor(
                            out=acc_t[s * F : (s + 1) * F, off : off + n],
                            in0=acc_t[s * F : (s + 1) * F, off : off + n],
                            in1=op_t[s * F : (s + 1) * F, 0:n],
                            op=mybir.AluOpType.add,
                        )
                for w in range(NW):
                    # ---- table window build: g = W_stack^T @ x_pre[window]
                    gw_t = gpool.tile([128, W + 1], f32, tag="gw")
                    nc.vector.memset(gw_t[:, W : W + 1], 0.0)
                    for k in range(W // 500):
                        xc_t = xpool.tile([128, 500], bf16, tag="x")
                        nc.sync.dma_start(
                            out=xc_t[:],
                            in_=xp_d[:, w * W + k * 500 : w * W + (k + 1) * 500],
                        )
                        gp_t = psA.tile([128, 500], f32, tag="gp")
                        nc.tensor.matmul(
                            gp_t[:], ws_t[:], xc_t[:], start=True, stop=True
                        )
                        nc.scalar.activation(
                            gw_t[:, k * 500 : (k + 1) * 500],
                            gp_t[:],
                            mybir.ActivationFunctionType.Copy,
                        )

                    # ---- indices for this window
                    im_t = ipool.tile([128, NIDX // 16], i16, tag="im")
                    ia_t = ipool.tile([128, R2 // 16], i16, tag="ia")
                    ib_t = ipool.tile([128, R1 // 16], i16, tag="ib")
                    nc.sync.dma_start(out=im_t[:], in_=im_d[w])
                    nc.sync.dma_start(out=ia_t[:], in_=ia_d[w])
                    nc.sync.dma_start(out=ib_t[:], in_=ib_d[w])

                    # ---- gather halves + slot reduce
                    for h, (s0, ns) in enumerate([(0, H1S), (H1S, H2S)]):
                        go_t = opool.tile([128, max(H1S, H2S) * SLOT], f32, tag="go")
                        nidx = ns * SLOT
                        nc.gpsimd.ap_gather(
                            go_t[:, 0:nidx].unsqueeze(2),
                            gw_t[:].unsqueeze(2),
                            im_t[:, s0 * SLOT // 16 : (s0 + ns) * SLOT // 16],
                            channels=128,
                            num_elems=W + 1,
                            d=1,
                            num_idxs=nidx,
                        )
                        v = go_t[:, 0:nidx].rearrange(
                            "p (n s) -> p n s", n=ns
                        )
                        nc.vector.tensor_tensor(
                            out=ss_t[:, s0 : s0 + ns].unsqueeze(2),
                            in0=v[:, :, 0:1],
                            in1=v[:, :, 1:2],
                            op=mybir.AluOpType.add,
                        )
                        nc.vector.tensor_tensor(
                            out=ss_t[:, s0 : s0 + ns].unsqueeze(2),
                            in0=ss_t[:, s0 : s0 + ns].unsqueeze(2),
                            in1=v[:, :, 2:3],
                            op=mybir.AluOpType.add,
                        )

                    # ---- cascade A: R3 sums -> R2 slot positions
                    ca_t = capool.tile([128, R2], f32, tag="ca")
                    nc.gpsimd.ap_gather(
                        ca_t[:].unsqueeze(2),
                        ss_t[:, R1 + R2 : NSLOTS + 1].unsqueeze(2),
                        ia_t[:],
                        channels=128,
                        num_elems=R3 + SP + 1,
                        d=1,
                        num_idxs=R2,
                    )
                    nc.vector.tensor_tensor(
                        out=ss_t[:, R1 : R1 + R2],
                        in0=ss_t[:, R1 : R1 + R2],
                        in1=ca_t[:],
                        op=mybir.AluOpType.add,
                    )
                    # ---- cascade B: (R2+R3) sums -> canonical positions
                    cb_t = capool.tile([128, R1], f32, tag="cb")
                    nc.gpsimd.ap_gather(
                        cb_t[:].unsqueeze(2),
                        ss_t[:, R1 : NSLOTS + 1].unsqueeze(2),
                        ib_t[:],
                        channels=128,
                        num_elems=R2 + R3 + SP + 1,
                        d=1,
                        num_idxs=R1,
                    )
                    nc.vector.tensor_tensor(
                        out=acc_t[:], in0=acc_t[:], in1=ss_t[:, 0:R1],
                        op=mybir.AluOpType.add,
                    )
                    nc.vector.tensor_tensor(
                        out=acc_t[:], in0=acc_t[:], in1=cb_t[:],
                        op=mybir.AluOpType.add,
                    )
                    # ---- stash spill slot sums
                    nc.vector.tensor_copy(
                        out=sp_t[:, w * SP : (w + 1) * SP],
                        in_=ss_t[:, R1 + R2 + R3 : NSLOTS],
                    )
                    if dumps:
                        nc.sync.dma_start(out=ssD[w], in_=ss_t[:])
                        if w == NW - 1:
                            nc.sync.dma_start(out=gwD[:], in_=gw_t[:])

                # ---- spill rounds
                for r in range(n_rounds):
                    is_t = ipool.tile([128, R1 // 16], i16, tag="isp")
                    nc.sync.dma_start(out=is_t[:], in_=isp_d[r])
                    sg_t = capool.tile([128, R1], f32, tag="cb")
                    nc.gpsimd.ap_gather(
                        sg_t[:].unsqueeze(2),
                        sp_t[:].unsqueeze(2),
                        is_t[:],
                        channels=128,
                        num_elems=NW * SP + 1,
                        d=1,
                        num_idxs=R1,
                    )
                    nc.vector.tensor_tensor(
                        out=acc_t[:], in0=acc_t[:], in1=sg_t[:],
                        op=mybir.AluOpType.add,
                    )

            if dumps:
                nc.sync.dma_start(out=spD[:], in_=sp_t[:])
                nc.sync.dma_start(out=accD[:], in_=acc_t[:])
            # ------------------------------------------------ epilogue
            with tc.tile_pool(name="epi", bufs=1) as epool:
                wd_t = epool.tile([128, 128], f32)
                eye_t = epool.tile([128, 128], f32)
                dv_t = epool.tile([128, R1], f32)
                nc.sync.dma_start(out=wd_t[:], in_=wd_d[:])
                nc.sync.dma_start(out=eye_t[:], in_=eye_d[:])
                nc.sync.dma_start(out=dv_t[:], in_=dv_d[:])
                nc.vector.tensor_tensor(
                    out=acc_t[:], in0=acc_t[:], in1=dv_t[:],
                    op=mybir.AluOpType.mult,
                )
                h1_t = epool.tile([128, R1], f32)
                nc.scalar.activation(
                    h1_t[:], acc_t[:],
                    mybir.ActivationFunctionType.Relu, bias=bg_t[:],
                )
                h2_t = epool.tile([128, OUTR], f32)
                nc.vector.memset(h2_t[:, R1:OUTR], 0.0)
                for k in range(R1 // 448):  # 3136 = 7*448
                    hp_t = psA.tile([128, 448], f32, tag="hp")
                    nc.tensor.matmul(
                        hp_t[:], wd_t[:],
                        h1_t[:, k * 448 : (k + 1) * 448],
                        start=True, stop=True,
                    )
                    nc.scalar.activation(
                        h2_t[:, k * 448 : (k + 1) * 448], hp_t[:],
                        mybir.ActivationFunctionType.Relu, bias=bd_t[:],
                    )
                ob_t = epool.tile([128, OUTR], f32)
                for t in range(OUTR // 128):
                    tp_t = psB.tile([128, 128], f32, tag="tp")
                    nc.tensor.transpose(
                        tp_t[:], h2_t[:, t * 128 : (t + 1) * 128], eye_t[:]
                    )
                    nc.scalar.activation(
                        ob_t[:, t * 128 : (t + 1) * 128], tp_t[:],
                        mybir.ActivationFunctionType.Copy,
                    )
                # out rows: [t, p, s, f] = ob[p, t*128 + s*32 + f]
                nc.sync.dma_start(
                    out=out_d.ap().rearrange(
                        "(t p s) f -> p t s f", p=128, s=NSTR
                    ),
                    in_=ob_t[:].rearrange("p (t s f) -> p t s f", s=NSTR, f=F),
                )
    nc.compile()
    return nc


# ------------------------------------------------------------- host prep


def host_prep(x, edge_index, W_gcn, b_gcn, W_dense, b_dense):
    x = np.ascontiguousarray(x, dtype=np.float32)
    W_gcn = np.ascontiguousarray(W_gcn, dtype=np.float32)
    W_dense = np.ascontiguousarray(W_dense, dtype=np.float32)
    b_gcn = np.ascontiguousarray(b_gcn, dtype=np.float32)
    b_dense = np.ascontiguousarray(b_dense, dtype=np.float32)
    row = np.asarray(edge_index[0], dtype=np.int64)
    col = np.asarray(edge_index[1], dtype=np.int64)

    deg = np.bincount(col, minlength=N).astype(np.float64) + 1.0
    dinv = (1.0 / np.sqrt(deg)).astype(np.float32)

    # self-loops are NOT routed through the gather: their contribution
    # (table[v] itself) is recomputed in the epilogue from xo = own x slice.

    # shared tensors
    xp = np.ascontiguousarray((x * dinv[:, None]).T.astype(BF16))  # [128, N]
    ws = np.zeros((128, 128), dtype=np.float32)
    for s in range(NSTR):
        ws[:, s * F : (s + 1) * F] = W_gcn
    ws = ws.astype(BF16)
    wd = np.zeros((128, 128), dtype=np.float32)
    for s in range(NSTR):
        wd[s * F : (s + 1) * F, s * F : (s + 1) * F] = W_dense
    bg = np.tile(b_gcn.reshape(F), NSTR).reshape(128, 1).astype(np.float32)
    bd = np.tile(b_dense.reshape(F), NSTR).reshape(128, 1).astype(np.float32)
    eye = np.eye(128, dtype=np.float32)

    # per-edge decomposition
    core = col // NLOC
    rem = col % NLOC
    stream = rem // DPS
    j = rem % DPS
    win = row // W
    soff = (row % W).astype(np.int64)

    # group edges by (core, stream, win, j)
    key = ((core * NSTR + stream) * NW + win) * DPS + j
    order = np.argsort(key, kind="stable")
    ks = key[order]
    so = soff[order]
    uq, starts, cnts = np.unique(ks, return_index=True, return_counts=True)
    nu = len(uq)
    ne = len(ks)
    rank = np.arange(ne) - np.repeat(starts, cnts)
    level = rank // SLOT
    pos = rank % SLOT

    grp = uq // DPS                  # (core, stream, win) id per unique key
    ju = uq % DPS
    assert cnts.max() <= 9 + SP * SLOT, cnts.max()

    def region_rank(mask):
        """rank of masked uq entries within their grp (ascending j)."""
        g = grp[mask]
        nm = int(mask.sum())
        if nm == 0:
            return np.zeros(0, dtype=np.int64)
        _, gstart, gcnt = np.unique(g, return_index=True, return_counts=True)
        return np.arange(nm) - np.repeat(gstart, gcnt)

    m2 = cnts >= SLOT + 1
    m3 = cnts >= 2 * SLOT + 1
    msp = cnts >= 3 * SLOT + 1
    r2rank = np.full(nu, -1, dtype=np.int64)
    r3rank = np.full(nu, -1, dtype=np.int64)
    sprank = np.full(nu, -1, dtype=np.int64)
    r2rank[m2] = region_rank(m2)
    r3rank[m3] = region_rank(m3)
    # spill slots: per uq, ceil((cnt-9)/3) slots; base = cumsum within grp
    spslots = np.where(msp, -(-(cnts - 3 * SLOT) // SLOT), 0)
    spbase = np.full(nu, -1, dtype=np.int64)
    if msp.any():
        g = grp[msp]
        v = spslots[msp]
        _, gstart, gcnt = np.unique(g, return_index=True, return_counts=True)
        cs = np.cumsum(v) - v
        base_per_grp = cs[gstart]
        spbase[msp] = cs - np.repeat(base_per_grp, gcnt)
    # capacity asserts (per grp)
    ngrp = NCORES * NSTR * NW
    r2cnt = np.bincount(grp[m2], minlength=ngrp)
    r3cnt = np.bincount(grp[m3], minlength=ngrp)
    spcnt = np.bincount(
        grp[msp], weights=spslots[msp].astype(np.float64), minlength=ngrp
    ).astype(np.int64)
    assert r2cnt.max() <= R2, r2cnt.max()
    assert r3cnt.max() <= R3, r3cnt.max()
    assert spcnt.max() <= SP, spcnt.max()

    # per-edge slot id within the NSLOTS space
    uq_of_edge = np.repeat(np.arange(nu), cnts)
    slot = np.empty(ne, dtype=np.int64)
    l0 = level == 0
    l1 = level == 1
    l2 = level == 2
    l3 = level >= 3
    slot[l0] = ju[uq_of_edge[l0]]
    slot[l1] = R1 + r2rank[uq_of_edge[l1]]
    slot[l2] = R1 + R2 + r3rank[uq_of_edge[l2]]
    slot[l3] = (
        R1 + R2 + R3 + spbase[uq_of_edge[l3]] + (level[l3] - 3)
    )
    idxpos = slot * SLOT + pos

    # main gather idx array
    cu = uq // (NW * DPS * NSTR)
    wu = (uq // DPS) % NW
    su = (uq // (NW * DPS)) % NSTR
    cu_e = np.repeat(cu, cnts)
    wu_e = np.repeat(wu, cnts)
    su_e = np.repeat(su, cnts)
    im = np.full((NCORES, NW, NSTR, NIDX), Z_WIN, dtype=np.int16)
    im[cu_e, wu_e, su_e, idxpos] = so.astype(np.int16)

    # cascade A: for each R2 slot -> its dest's R3 slot (or zero)
    ia = np.full((NCORES, NW, NSTR, R2), Z_CA, dtype=np.int16)
    ia[cu[m2], wu[m2], su[m2], r2rank[m2]] = np.where(
        m3[m2], r3rank[m2], Z_CA
    ).astype(np.int16)
    # cascade B: canonical j -> R2 slot (or zero)
    ib = np.full((NCORES, NW, NSTR, R1), Z_CB, dtype=np.int16)
    ib[cu[m2], wu[m2], su[m2], ju[m2]] = r2rank[m2].astype(np.int16)

    # spill final rounds: per (core, stream, j) pooled spill slots
    isp_list = []
    if msp.any():
        su_s = su[msp]
        cu_s = cu[msp]
        wu_s = wu[msp]
        ju_s = ju[msp]
        nsl = spslots[msp]
        sb = spbase[msp]
        # expand each uq's spill slots
        reps = np.repeat(np.arange(len(nsl)), nsl)
        k = np.arange(len(reps)) - np.repeat(
            np.cumsum(nsl) - nsl, nsl
        )
        gpos = wu_s[reps] * SP + sb[reps] + k      # pos in spill buffer
        ckey = (cu_s[reps] * NSTR + su_s[reps]) * DPS + ju_s[reps]
        o2 = np.argsort(ckey, kind="stable")
        ck = ckey[o2]
        gp = gpos[o2]
        _, cst, ccn = np.unique(ck, return_index=True, return_counts=True)
        rr = np.arange(len(ck)) - np.repeat(cst, ccn)
        n_rounds = int(ccn.max())
        isp = np.full((NCORES, n_rounds, NSTR, R1), Z_SP_BUF, dtype=np.int16)
        isp[ck // (NSTR * DPS), rr, (ck // DPS) % NSTR, ck % DPS] = gp.astype(
            np.int16
        )
    else:
        n_rounds = 0
        isp = np.full((NCORES, 1, NSTR, R1), Z_SP_BUF, dtype=np.int16)

    # wrap idx arrays
    im_w = np.stack(
        [
            _wrap_subgathers(im[c], [H1S * SLOT, H2S * SLOT])
            for c in range(NCORES)
        ]
    )
    ia_w = np.stack([_wrap(ia[c]) for c in range(NCORES)])
    ib_w = np.stack([_wrap(ib[c]) for c in range(NCORES)])
    isp_w = np.stack([_wrap(isp[c]) for c in range(NCORES)])

    # dinv per dest in canonical layout
    dv = np.zeros((NCORES, 128, R1), dtype=np.float32)
    for c in range(NCORES):
        for s in range(NSTR):
            seg = dinv[c * NLOC + s * DPS : c * NLOC + (s + 1) * DPS]
            dv[c, s * F : (s + 1) * F, 0:DPS] = seg[None, :]

    in_maps = []
    for c in range(NCORES):
        in_maps.append(
            {
                "xp": xp,
                "xo": np.ascontiguousarray(xp[:, c * NLOC : (c + 1) * NLOC]),
                "ws": ws,
                "wd": wd,
                "bg": bg,
                "bd": bd,
                "eye": eye,
                "dv": dv[c],
                "im": im_w[c],
                "ia": ia_w[c],
                "ib": ib_w[c],
                "isp": isp_w[c],
            }
        )
    return in_maps, n_rounds


def assemble_out(results):
    """results[c]["out"]: [OUTR*NSTR, F] rows = [t, p, s, f] ->
    dest j = t*128+p of stream s."""
    outs = []
    for c in range(NCORES):
        o = results[c]["out"].reshape(OUTR // 128, 128, NSTR, F)
        o = o.transpose(2, 0, 1, 3).reshape(NSTR * OUTR, F)
        outs.append(
            np.concatenate(
                [o[s * OUTR : s * OUTR + DPS] for s in range(NSTR)]
            )
        )
    return np.concatenate(outs, axis=0)


_NC_CACHE = {}


def _get_nc(n_rounds, num_devices=NCORES):
    key = (n_rounds, num_devices)
    if key not in _NC_CACHE:
        _NC_CACHE[key] = build_nc(n_rounds, num_devices)
    return _NC_CACHE[key]


def kernel(x, edge_index, W_gcn, b_gcn, W_dense, b_dense):
    in_maps, n_rounds = host_prep(
        x, edge_index, W_gcn, b_gcn, W_dense, b_dense
    )
    nc = _get_nc(n_rounds)
    core_ids = list(range(NCORES))
    res = run_bass_kernel_spmd(nc, in_maps, core_ids).results
    return assemble_out(res)
